# revision 1
# baseline (speedup 1.0000x reference)
"""AdEx neuron simulation kernel for 8 Trainium2 NeuronCores.

Reference semantics (per timestep, fp32):
    exp_term = Delta_T * exp((V - V_T)/Delta_T)
    V <- V + dt/tau_m * (-(V-E_L) + exp_term - R*w + R*I)
    spk = V >= V_spike ; V <- V_reset where spk
(w stays identically 0 for the a=0, b=0 parameterization.)

Kernel formulation (state Y = V - V_reset, c = dt/tau_m, A = 1-c):
    e_t = exp(s*Y + b)            s = 1/Delta_T, b = (V_reset-V_T)/Delta_T + ln(c*Delta_T)
    u_t = A*Y_{t-1} + J_t + e_t   J_t = c*R*I_t + c*(E_L - V_reset)   (host-precomputed)
    Y_t = u_t if u_t < thr else 0 thr = V_spike - V_reset
    spike_t = (Y_t == 0)          extracted in bulk per chunk

Sharding: batch rows 4k..4k+3 -> core k (4096 neurons/core, [128 x 32] tiles),
serial 2000-step loop per core; no cross-core communication.
Engines: ScalarE computes exp (PWP table); VectorE runs the recurrence as
three scalar_tensor_tensor ops per step (u = h + e on the critical chain,
select-reset, and the off-chain h_{t+1} = A*Y_t + J_{t+1} prefold that
overlaps the next exp). Spikes are extracted per step as (u_t >= thr) —
the reference's exact spike definition — with the small op hidden in the
DVE idle window while the next exp runs (a bulk per-chunk extraction
would stall the chain ~4us at every chunk tail). Chunks of 125 steps are
DMA'd out while the loop continues. All cross-engine waits are attached
to compute instructions (saves ~150ns/step vs standalone waits).
"""

import numpy as np

B, T, D = 32, 2000, 1024
N_CORES = 8
BPC = B // N_CORES            # batch rows per core
NPC = BPC * D                 # neurons per core = 4096
W = NPC // 128                # free-dim width = 32


def _build_graph(consts, G=1, CH=125, steps=T):
    import concourse.bass as bass
    import concourse.mybir as mybir

    A, s, bias, thr = consts["A"], consts["s"], consts["bias"], consts["thr"]
    y0 = consts["y0"]
    f32 = mybir.dt.float32
    NCH = steps // CH
    assert steps % CH == 0
    GW = W // G
    assert W % G == 0

    nc = bass.Bass()

    # init constants in SBUF
    bias_t = nc.alloc_sbuf_tensor("expbias", [128, 1], f32)
    nc.gpsimd.memset(bias_t.ap(), float(bias))
    yinit = nc.alloc_sbuf_tensor("yinit", [128, W], f32)
    nc.gpsimd.memset(yinit.ap(), float(y0))
    nc.all_engine_barrier()

    J_ext = nc.declare_dram_parameter("J", [128, steps, W], f32, isOutput=False)
    spk_ext = nc.declare_dram_parameter("spk", [128, steps, W], f32, isOutput=True)

    with (
        nc.sbuf_tensor([128, 2, CH, W], f32) as jbuf,
        nc.sbuf_tensor([128, 2, CH, W], f32) as hist,
        nc.sbuf_tensor([128, 2, CH, W], f32) as spkst,
        nc.sbuf_tensor([128, 2, W], f32) as ebuf,
        nc.sbuf_tensor([128, 2, W], f32) as hbuf,
        nc.sbuf_tensor([128, 2, W], f32) as ubuf,
        nc.semaphore("spk_sem") as spk_sem,
        nc.Block() as block,
    ):
        # ONE act/dve semaphore shared across groups (HW sem pool is small;
        # per-group sems at G>=2 alias and break the exp/select handshake).
        # In-order engines make cumulative counts map exactly to instruction
        # order: the k-th select in program order is the k-th increment.
        act_sem = nc.semaphore("act_sem").__enter__()
        dve_sem = nc.semaphore("dve_sem").__enter__()
        # per-parity DMA sems: completion increments of different DMAs can
        # arrive out of order, so each jbuf/spkst half gets its own semaphore
        dmaJ_sems = [nc.semaphore(f"dmaJ_sem{p}").__enter__() for p in range(2)]
        dmaS_sems = [nc.semaphore(f"dmaS_sem{p}").__enter__() for p in range(2)]

        def gsl(g):
            return slice(g * GW, (g + 1) * GW)

        def yprev(t, g):
            if t == 0:
                return yinit.ap()[:, gsl(g)]
            tm = t - 1
            return hist[:, (tm // CH) % 2, tm % CH, gsl(g)]

        @block.sync
        def _(sync):
            # prefetch the first two J chunks
            for ci in range(min(2, NCH)):
                sync.dma_start(
                    jbuf[:, ci % 2], J_ext[:, ci * CH:(ci + 1) * CH]
                ).then_inc(dmaJ_sems[ci % 2], 16)
            for ci in range(NCH):
                # write back spike chunk ci once extracted
                sync.dma_start(
                    spk_ext[:, ci * CH:(ci + 1) * CH], spkst[:, ci % 2]
                )._wait_ge(spk_sem, CH * (ci + 1)).then_inc(dmaS_sems[ci % 2], 16)
                # prefetch J chunk ci+2 (reuses buffer of chunk ci, consumed
                # by the time DVE's chunk-ci spikes are extracted)
                if ci + 2 < NCH:
                    sync.dma_start(
                        jbuf[:, ci % 2], J_ext[:, (ci + 2) * CH:(ci + 3) * CH]
                    ).then_inc(dmaJ_sems[ci % 2], 16)

        @block.scalar
        def _(scalar):
            for t in range(steps):
                for g in range(G):
                    ins = nc.scalar.activation(
                        ebuf[:, t % 2, gsl(g)], yprev(t, g),
                        mybir.ActivationFunctionType.Exp,
                        bias=bias_t.ap(), scale=float(s),
                    ).then_inc(act_sem, 1)
                    if t >= 1:
                        # needs select(t-1, g) = inc number G*(t-1)+g+1
                        ins._wait_ge(dve_sem, G * (t - 1) + g + 1)

        @block.vector
        def _(vector):
            # h_0 prologue: h[0] = A*yinit + J_0
            nc.vector.scalar_tensor_tensor(
                hbuf[:, 0], yinit.ap(), float(A), jbuf[:, 0, 0],
                op0=mybir.AluOpType.mult, op1=mybir.AluOpType.add,
            )._wait_ge(dmaJ_sems[0], 16)
            for t in range(steps):
                ci = t // CH
                for g in range(G):
                    # on-chain: u_t = h_t + e_t ; Y_t = (u_t < thr) * u_t
                    nc.vector.scalar_tensor_tensor(
                        ubuf[:, t % 2, gsl(g)], hbuf[:, t % 2, gsl(g)], 0.0,
                        ebuf[:, t % 2, gsl(g)],
                        op0=mybir.AluOpType.add, op1=mybir.AluOpType.add,
                    )._wait_ge(act_sem, G * t + g + 1)
                    nc.vector.scalar_tensor_tensor(
                        hist[:, ci % 2, t % CH, gsl(g)],
                        ubuf[:, t % 2, gsl(g)], float(thr),
                        ubuf[:, t % 2, gsl(g)],
                        op0=mybir.AluOpType.is_lt, op1=mybir.AluOpType.mult,
                    ).then_inc(dve_sem, 1)
                    # off-chain: h_{t+1} = A*Y_t + J_{t+1}
                    if t + 1 < steps:
                        tn = t + 1
                        cn = tn // CH
                        ins = nc.vector.scalar_tensor_tensor(
                            hbuf[:, tn % 2, gsl(g)],
                            hist[:, ci % 2, t % CH, gsl(g)], float(A),
                            jbuf[:, cn % 2, tn % CH, gsl(g)],
                            op0=mybir.AluOpType.mult, op1=mybir.AluOpType.add,
                        )
                        if tn % CH == 0:
                            ins._wait_ge(dmaJ_sems[cn % 2], 16 * (cn // 2 + 1))
                # per-step spike extraction: spk_t = (u_t >= thr), the
                # reference's exact spike definition. A small op here hides in
                # the DVE idle window while the next exp runs; the old bulk
                # per-chunk extraction ([128, CH*W], ~4.2us) stalled the next
                # u-add behind it at every chunk tail (~61us total).
                ins = nc.vector.tensor_scalar(
                    spkst[:, ci % 2, t % CH], ubuf[:, t % 2], float(thr), None,
                    mybir.AluOpType.is_ge,
                ).then_inc(spk_sem, 1)
                if t % CH == 0 and ci >= 2:
                    # don't overwrite spkst half still being DMA'd out
                    ins._wait_ge(dmaS_sems[ci % 2], 16 * ((ci - 2) // 2 + 1))

    return nc


def _derive_consts(params):
    tau_m, E_L, V_T, Delta_T, R, tau_w, a, b, V_reset, V_spike, dt = [
        float(x) for x in params
    ]
    c = dt / tau_m
    return dict(
        A=np.float32(1.0 - c),
        s=np.float32(1.0 / Delta_T),
        bias=np.float32(np.log(c * Delta_T) + (V_reset - V_T) / Delta_T),
        thr=np.float32(V_spike - V_reset),
        y0=np.float32(E_L - V_reset),
        cR=np.float32(c * R),
        Jc=np.float32(c * (E_L - V_reset)),
        a=a, b=b,
    )


def _numpy_fallback(I_seq, params):
    # general-parameter reference port (slow, CPU); used only if a != 0 or b != 0
    tau_m, E_L, V_T, Delta_T, R, tau_w, a, b, V_reset, V_spike, dt = [
        np.float32(x) for x in params
    ]
    Bs, Ts, Ds = I_seq.shape
    I = I_seq.transpose(1, 0, 2).reshape(Ts, Bs * Ds)
    V = np.full(Bs * Ds, E_L, dtype=np.float32)
    w = np.zeros(Bs * Ds, dtype=np.float32)
    out = np.zeros((Ts, Bs * Ds), dtype=np.float32)
    for t in range(Ts):
        exp_term = Delta_T * np.exp((V - V_T) / Delta_T)
        dV = (-(V - E_L) + exp_term - R * w + R * I[t]) / tau_m
        V = V + dt * dV
        dw = (a * (V - E_L) - w) / tau_w
        w = w + dt * dw
        spk = (V >= V_spike).astype(np.float32)
        V = np.where(spk > 0, V_reset, V)
        w = np.where(spk > 0, w + b, w)
        out[t] = spk
    return out.reshape(Ts, Bs, Ds).transpose(1, 0, 2)


_CACHE = {}


def kernel(I_seq, params):
    I_seq = np.asarray(I_seq, dtype=np.float32)
    params = np.asarray(params, dtype=np.float32)
    consts = _derive_consts(params)
    if consts["a"] != 0.0 or consts["b"] != 0.0:
        return _numpy_fallback(I_seq, params)

    from concourse.bass_utils import run_bass_kernel_spmd

    # host-side input prep: J = cR*I + Jc, laid out [128, T, 32] per core
    J = (consts["cR"] * I_seq + consts["Jc"]).astype(np.float32)
    eye = np.eye(128, dtype=np.float32)
    in_maps = []
    for k in range(N_CORES):
        jk = J[BPC * k: BPC * (k + 1)]                       # [4, T, 1024]
        jk = jk.reshape(BPC, T, W, D // W // 1)              # [4, T, 32, 32]
        jk = np.ascontiguousarray(jk.transpose(0, 2, 1, 3))  # [4, 32, T, 32]
        jk = jk.reshape(128, T, W)
        in_maps.append({"J": jk, "eye": eye})

    import os
    G = int(os.environ.get("ADEX_G", "1"))
    CH = int(os.environ.get("ADEX_CH", "125"))
    key = (np.asarray(params).tobytes(), G, CH)
    if key not in _CACHE:
        _CACHE[key] = _build_graph(consts, G=G, CH=CH)
    nc = _CACHE[key]

    res = run_bass_kernel_spmd(nc, in_maps, core_ids=list(range(N_CORES)))

    out = np.empty((B, T, D), dtype=np.float32)
    for k in range(N_CORES):
        sk = res.results[k]["spk"]                           # [128, T, 32]
        sk = sk.reshape(BPC, W, T, D // W)                   # [4, 32, T, 32]
        sk = sk.transpose(0, 2, 1, 3).reshape(BPC, T, D)     # [4, T, 1024]
        out[BPC * k: BPC * (k + 1)] = sk
    return out

